# revision 1
# baseline (speedup 1.0000x reference)
"""Swin-style window attention kernel for 8 TRN2 NeuronCores (SPMD, batch-sharded).

Layout strategy per core (16 windows):
  - xT via PE transpose; qkv projection in float32r (N=392 token-pairs).
  - q,k kept feature-major [d,tok] bf16; v token-major [tok, (h,d)] bf16.
  - Per head: QK^T transposed (attnT [k,196] psum) via 32-row-packed matmuls,
    exp on ACT -> bf16, * exp(bias) gathered on-device via dma_gather,
    AV dense M=32 (4 heads/bank), softmax denominators via ones-matmuls,
    reciprocal broadcast via a selection matmul, normalize fused into evac.
  - proj in float32r, bias added during psum evacuation.
"""
import numpy as np

B, NT, CH = 128, 196, 512
H, D = 16, 32
NH4 = 4            # heads per group
NCORES = 8
WPC = B // NCORES  # windows per core
KSLOTS = 256       # padded k slots for the bias gather
NIDX = NT * KSLOTS  # 50176
TABLE_N = 729

_CACHE = {}


def _build():
    import concourse.bass as bass
    import concourse.mybir as mybir
    import concourse.tile as tile
    from concourse import bacc
    from concourse.masks import make_identity

    fp32 = mybir.dt.float32
    f32r = mybir.dt.float32r
    bf16 = mybir.dt.bfloat16
    fp16 = mybir.dt.float16
    i16 = mybir.dt.int16
    AF = mybir.ActivationFunctionType

    nc = bacc.Bacc("TRN2", target_bir_lowering=False, debug=False, num_devices=NCORES)

    x = nc.dram_tensor("x", [WPC, NT, CH], fp32, kind="ExternalInput")
    qkv_w = nc.dram_tensor("qkv_w", [CH, 3 * CH], fp32, kind="ExternalInput")
    biast = nc.dram_tensor("biast", [128, H, 2 * NT], fp32, kind="ExternalInput")
    proj_w = nc.dram_tensor("proj_w", [CH, CH], fp32, kind="ExternalInput")
    proj_b = nc.dram_tensor("proj_b", [1, CH], fp32, kind="ExternalInput")
    y = nc.dram_tensor("y", [WPC, NT, CH], fp32, kind="ExternalOutput")
    import os as _os
    _dbg = _os.environ.get("KDEBUG") == "1"
    if _dbg:
        d_xt = nc.dram_tensor("d_xt", [128, 4, 2 * NT], fp32, kind="ExternalOutput")
        d_qk = nc.dram_tensor("d_qk", [128, 8, 2 * NT + 60], bf16, kind="ExternalOutput")
        d_v = nc.dram_tensor("d_v", [128, 2, H, D], bf16, kind="ExternalOutput")
        d_ebt = nc.dram_tensor("d_ebt", [128, H, 2 * NT], bf16, kind="ExternalOutput")
        d_et = nc.dram_tensor("d_et", [128, 4, 2 * NT], bf16, kind="ExternalOutput")
        d_ar = nc.dram_tensor("d_ar", [128, 4, NT], fp32, kind="ExternalOutput")

    with tile.TileContext(nc) as tc:
        with (
            tc.tile_pool(name="const", bufs=1) as cpool,
            tc.tile_pool(name="dram", bufs=1, space="DRAM") as dpool,
            tc.tile_pool(name="work", bufs=2) as wpool,
            tc.tile_pool(name="attn", bufs=3) as apool,
            tc.tile_pool(name="ps_qk", bufs=1, space="PSUM") as ps_qk,
            tc.tile_pool(name="ps_sm", bufs=4, space="PSUM") as ps_sm,
        )  :
            # ---------------- one-time setup ----------------
            ident = cpool.tile([128, 128], fp32)
            make_identity(nc, ident)

            # weights, rounded to f32r
            wq32 = cpool.tile([128, 4, 3 * CH], fp32, tag="wq32")
            nc.sync.dma_start(wq32[:], qkv_w.ap().rearrange("(ko ki) m -> ki ko m", ki=128))
            wq = cpool.tile([128, 4, 3 * CH], f32r, tag="wq")
            nc.vector.tensor_copy(wq[:], wq32[:])

            pw32 = cpool.tile([128, 4, CH], fp32, tag="pw32")
            nc.sync.dma_start(pw32[:], proj_w.ap().rearrange("(ko ki) m -> ki ko m", ki=128))
            pw = cpool.tile([128, 4, CH], f32r, tag="pw")
            nc.vector.tensor_copy(pw[:], pw32[:])

            # proj_b broadcast to 128 partitions
            b_row = cpool.tile([1, CH], fp32, tag="brow")
            nc.sync.dma_start(b_row[:], proj_b.ap())
            b_bcast = cpool.tile([128, CH], fp32, tag="bb")
            nc.gpsimd.partition_broadcast(b_bcast[:], b_row[:], channels=128)

            # exp(bias) from host-gathered biasT
            bt_in = cpool.tile([128, H, 2 * NT], fp32, tag="btin")
            nc.sync.dma_start(bt_in[:], biast.ap())
            ebt = cpool.tile([128, H, 2 * NT], bf16, tag="ebt")
            nc.scalar.activation(ebt[:], bt_in[:], AF.Exp)

            ones32 = cpool.tile([128, 32], mybir.dt.bfloat16, tag="ones")
            nc.gpsimd.memset(ones32[:], 1.0)

            if _dbg:
                nc.sync.dma_start(d_ebt.ap(), ebt[:])

            # ---------------- main loop ----------------
            for pair in range(WPC // 2):
                # -------- pair stage: xT, qkv --------
                xT = wpool.tile([128, 4, 2 * NT], f32r, tag="xT")
                for wi in range(2):
                    w = 2 * pair + wi
                    wo = wi * NT
                    xa = wpool.tile([128, CH], fp32, tag="xa")
                    nc.sync.dma_start(xa[:], x.ap()[w, 0:128, :])
                    xb = wpool.tile([68, CH], fp32, tag="xb")
                    nc.sync.dma_start(xb[:], x.ap()[w, 128:NT, :])
                    tpa = ps_sm.tile([128, 512], fp32, tag="ps", name="tpa").rearrange("p (b c) -> p b c", b=4)
                    tpc = ps_sm.tile([128, 512], fp32, tag="ps", name="tpc").rearrange("p (b c) -> p b c", b=4)
                    for kc in range(4):
                        nc.tensor.transpose(tpa[:, kc, :], xa[:, kc * 128:(kc + 1) * 128], ident[:])
                        nc.tensor.transpose(tpc[:, kc, 0:68], xb[:, kc * 128:(kc + 1) * 128], ident[0:68, 0:68])
                    nc.vector.tensor_copy(xT[:, :, wo:wo + 128], tpa[:])
                    nc.vector.tensor_copy(xT[:, :, wo + 128:wo + NT], tpc[:, :, 0:68])

                # q,k feature-major [128, blk, 392] bf16
                qk = wpool.tile([128, 8, 2 * NT + 60], bf16, tag="qk")
                nc.gpsimd.memset(qk[:, :, 2 * NT:], 0.0)
                for mb in range(8):
                    qpv = ps_sm.tile([128, 512], fp32, tag="ps", name="qpv")
                    for kc in range(4):
                        nc.tensor.matmul(qpv[:, 0:2 * NT], wq[:, kc, mb * 128:(mb + 1) * 128],
                                         xT[:, kc, :], start=(kc == 0), stop=(kc == 3))
                    nc.scalar.activation(qk[:, mb, 0:2 * NT], qpv[:, 0:2 * NT], AF.Copy)

                # v token-major [128(tok), 2(chunk), H, D] bf16, per window
                vs = [None, None]
                for wi in range(2):
                    wo = wi * NT
                    v_sb = wpool.tile([128, 2, H, D], bf16, tag=f"v{wi}")
                    vs[wi] = v_sb
                    for tch, tsz in ((0, 128), (1, 68)):
                        vpv = ps_sm.tile([128, 512], fp32, tag="ps", name="vpv")
                        for kc in range(4):
                            nc.tensor.matmul(
                                vpv[0:tsz, 0:CH],
                                xT[:, kc, wo + tch * 128: wo + tch * 128 + tsz],
                                wq[:, kc, 2 * CH:3 * CH],
                                start=(kc == 0), stop=(kc == 3))
                        nc.scalar.activation(v_sb[0:tsz, tch, :, :].rearrange("p h d -> p (h d)"),
                                             vpv[0:tsz, 0:CH], AF.Copy)

                if _dbg and pair == 0:
                    nc.sync.dma_start(d_xt.ap(), xT.bitcast(fp32)[:])
                    nc.sync.dma_start(d_qk.ap(), qk[:])
                    nc.sync.dma_start(d_v.ap()[:, 0], vs[0][:, 0])
                    nc.sync.dma_start(d_v.ap()[0:68, 1], vs[0][0:68, 1])

                # -------- per-window attention --------
                import os as _os
                _stage = _os.environ.get("KSTAGE", "full")
                for wi in range(2 if _stage != "qkv" else 0):
                    w = 2 * pair + wi
                    wo = wi * NT
                    v_sb = vs[wi]
                    attn_r = apool.tile([128, 4, NT], f32r, tag="attn_r")

                    for g in range(4):
                        qkps = ps_qk.tile([128, 4, 512], fp32, tag="qkps")
                        for j in range(NH4):
                            h = 4 * g + j
                            hb = 32 * (h % 4)
                            qblk, kblk = h // 4, 4 + h // 4
                            rhs_q = qk[hb:hb + 32, qblk, wo:wo + NT]
                            nc.tensor.matmul(qkps[:, j, 0:NT],
                                             qk[hb:hb + 32, kblk, wo:wo + 128],
                                             rhs_q, start=True, stop=True,
                                             tile_position=(hb, 0))
                            nc.tensor.matmul(qkps[:, j, NT:2 * NT],
                                             qk[hb:hb + 32, kblk, wo + 128:wo + 256],
                                             rhs_q, start=True, stop=True,
                                             tile_position=(hb, 0))
                        esb = apool.tile([128, 4, 2 * NT], bf16, tag="esb")
                        nc.scalar.activation(esb[:], qkps[:, :, 0:2 * NT], AF.Exp)
                        et = apool.tile([128, 4, 2 * NT], bf16, tag="et")
                        nc.vector.tensor_mul(et[:], esb[:], ebt[:, 4 * g:4 * g + 4, :])
                        # AV dense (bank 0) + replicated denominators (bank 1)
                        avps = ps_sm.tile([128, 512], fp32, tag="ps", name="avps")
                        for j in range(NH4):
                            h = 4 * g + j
                            nc.tensor.matmul(avps[32 * j:32 * j + 32, 0:NT],
                                             v_sb[:, 0, h, :], et[:, j, 0:NT],
                                             start=True, stop=False,
                                             tile_position=(0, 32 * j))
                            nc.tensor.matmul(avps[32 * j:32 * j + 32, 0:NT],
                                             v_sb[0:68, 1, h, :], et[0:68, j, NT:2 * NT],
                                             start=False, stop=True,
                                             tile_position=(0, 32 * j))
                            nc.tensor.matmul(avps[32 * j:32 * j + 32, 256:256 + NT],
                                             ones32[:], et[:, j, 0:NT],
                                             start=True, stop=False,
                                             tile_position=(0, 32 * j))
                            nc.tensor.matmul(avps[32 * j:32 * j + 32, 256:256 + NT],
                                             ones32[0:68, :], et[0:68, j, NT:2 * NT],
                                             start=False, stop=True,
                                             tile_position=(0, 32 * j))
                        if _dbg and w == 0 and g == 0:
                            nc.sync.dma_start(d_et.ap(), et[:])
                        r_d = apool.tile([128, NT], fp16, tag="rd")
                        with nc.allow_low_precision(reason="softmax recip in fp16 is plenty"):
                            nc.vector.reciprocal(r_d[:], avps[:, 256:256 + NT])
                        nc.vector.tensor_mul(attn_r[:, g, :], avps[:, 0:NT], r_d[:])

                    if _dbg and w == 0:
                        nc.sync.dma_start(d_ar.ap(), attn_r.bitcast(fp32)[:])

                    # projection + bias
                    for tch, tsz in (((0, 128), (1, 68)) if _stage != "noproj" else ()):
                        pp = ps_sm.tile([128, 512], fp32, tag="ps", name="pp")
                        for bl in range(4):
                            nc.tensor.matmul(pp[0:tsz, 0:CH],
                                             attn_r[:, bl, tch * 128:tch * 128 + tsz],
                                             pw[:, bl, :], start=(bl == 0), stop=(bl == 3))
                        yt = wpool.tile([128, CH], fp32, tag="yt")
                        nc.vector.tensor_add(yt[0:tsz, :], pp[0:tsz, 0:CH], b_bcast[0:tsz, :])
                        nc.gpsimd.dma_start(y.ap()[w, tch * 128:tch * 128 + tsz, :], yt[0:tsz, :])

    nc.compile()
    return nc


def _prep_biast(rel_pos_index, rel_bias_table):
    # biast[p, h, khi*196 + q] = table[idx[q, p + 128*khi], h]  (0 where k out of range)
    idx = np.asarray(rel_pos_index).astype(np.int64)
    table = np.asarray(rel_bias_table, dtype=np.float32)
    g = table[idx]                      # [q, k, H]
    out = np.zeros((256, H, NT), dtype=np.float32)
    out[:NT] = g.transpose(1, 2, 0)     # [k, H, q]
    return np.ascontiguousarray(
        out.reshape(2, 128, H, NT).transpose(1, 2, 0, 3).reshape(128, H, 2 * NT))


def kernel(x, qkv_w, rel_bias_table, proj_w, proj_b, rel_pos_index):
    from concourse.bass_utils import run_bass_kernel_spmd

    if "nc" not in _CACHE:
        _CACHE["nc"] = _build()
    nc = _CACHE["nc"]

    x = np.ascontiguousarray(np.asarray(x), dtype=np.float32)
    scale = float((CH // H) ** (-0.5))
    qkv_s = np.array(qkv_w, dtype=np.float32, copy=True)
    qkv_s[:, :CH] *= scale
    biast = _prep_biast(rel_pos_index, rel_bias_table)
    pw = np.ascontiguousarray(np.asarray(proj_w), dtype=np.float32)
    pb = np.ascontiguousarray(np.asarray(proj_b), dtype=np.float32).reshape(1, CH)

    in_maps = []
    for c in range(NCORES):
        in_maps.append({
            "x": x[c * WPC:(c + 1) * WPC],
            "qkv_w": qkv_s,
            "biast": biast,
            "proj_w": pw,
            "proj_b": pb,
        })
    res = run_bass_kernel_spmd(nc, in_maps, core_ids=list(range(NCORES)))
    out = np.concatenate([r["y"] for r in res.results], axis=0)
    return out.astype(np.float32)


if __name__ == "__main__":
    pass

